# revision 52
# baseline (speedup 1.0000x reference)
# Dilated causal self-attention kernel for Trainium2 (8 NeuronCores).
#
# Reference computation (see problem):
#   x (4, 8192, 1024) -> reshape (4, 4, 2048, 1024) -> take every 4th token
#   -> per-segment causal MHA (16 heads, dh=64) -> scatter back into zeros.
#
# Sharding: 16 independent (batch, segment) attention problems, 2 per core.
# Host does the dilated gather + transpose + bf16 cast and the final scatter
# into the zero background; each core runs QKV -> per-head causal softmax
# attention -> output projection on its 2 segments.
#
# Device layout (all feature-major where possible):
#   xiT    [C, M]  (per segment)         - input, bf16
#   qkT    [2C, M] feature-major         - q rows pre-scaled by 1/sqrt(dh)
#                                          (folded into w_in on host)
#   v      [M, C]  token-major           - v bias folds into output bias
#   scores [128 q, n k] per (head, q-chunk), n = (qc+1)*128 (causal skip)
#   p = exp(scores) (no max subtraction: scores ~ N(0,1)), accum_out = denom
#   PV: outT[dh, M] = sum_kc v_kc^T @ pT_kc   (pT via PE transpose)
#   yT = w_outT^T @ oT + b_out_eff  -> DMA out feature-major (bf16)
#
# Perf changes vs the 206us baseline (measured 177us):
#   - scores emitted in head PAIRS: even head on PE row-group 0, odd head
#     on row-group 64 (K=dh=64 < 128) -> the two 64-row matmuls execute
#     concurrently (PE row tiling), halving score matmul time.
#   - per-kc fused exp: both heads' score chunks land in one 2-bank PSUM
#     pair tile [128,2,M], so one scalar activation covers the pair
#     (ACT per-instruction overhead ~290ns was saturating the scalar
#     engine in the attention tail and re-throttling the PE HAM).
#   - normalization without partition broadcasts: each head's PV
#     stationary is [ones(64) | v(64)], so the PV matmul itself emits 64
#     replicated denominator rows next to the 64 output rows; the whole
#     softmax divide is one [64,M] reciprocal + one [64,M] multiply on
#     the DVE (no gpsimd broadcast, no [1,M] single-lane DVE ops).
#   - dense filler (seg1 QKV, seg0 proj) spread uniformly, 2 units per
#     attention iteration across both segments' attention phases, so the
#     PE span per iteration (~5us) always exceeds the busiest non-PE
#     engine (~4us) and the HAM stays at K=8/8.
#   - startup: x(0)/wqk(0) quarter-DMAs interleaved so the first QKV
#     units' slices land first; dummy warm-up matmuls on a memset tile
#     run during the DMA-wait window to pre-warm the PE HAM state.
#   - causal mask via gpsimd affine_select on contiguous [128,128]
#     halves; psum drains alternate scalar/vector; output DMAs as bf16.

import sys

sys.path.insert(0, "/opt/trn_rl_repo")

import numpy as np
import ml_dtypes

import concourse.bacc as bacc
import concourse.mybir as mybir
from concourse.tile import TileContext
from concourse.bass_utils import run_bass_kernel_spmd

BF16 = ml_dtypes.bfloat16

B, N, C = 4, 8192, 1024
W_SEG, RATE, H = 2048, 4, 16
DH = C // H            # 64
S = N // W_SEG         # 4 segments per batch
M = W_SEG // RATE      # 512 tokens per segment
N_CORES = 8
SEG_PER_CORE = (B * S) // N_CORES  # 2

FP32 = mybir.dt.float32
BF = mybir.dt.bfloat16

_CACHE = {}


def _build():
    nc = bacc.Bacc()
    phase_of = _CACHE.setdefault("phase_of", {})

    def mm(phase, *args, **kwargs):
        inst = nc.tensor.matmul(*args, **kwargs)
        try:
            phase_of[inst.ins.name] = phase
        except Exception:
            pass
        return inst
    # chunk-major packed layouts (one DMA each; see _prep_inputs)
    xiT = nc.dram_tensor("xiT", [SEG_PER_CORE * 128, 8 * M], BF, kind="ExternalInput")
    wqk = nc.dram_tensor("wqk", [4 * 128, 8 * 512], BF, kind="ExternalInput")
    wv = nc.dram_tensor("wv", [128, 8 * C], BF, kind="ExternalInput")
    wout = nc.dram_tensor("wout", [128, 8 * C], BF, kind="ExternalInput")
    bqk = nc.dram_tensor("bqk", [128, 16], FP32, kind="ExternalInput")
    bout = nc.dram_tensor("bout", [128, 8], FP32, kind="ExternalInput")
    yT = nc.dram_tensor("yT", [SEG_PER_CORE * 4 * 128, 2 * M], BF,
                    kind="ExternalOutput")

    CT = C // 128  # 8 contraction chunks

    from contextlib import ExitStack
    with TileContext(nc) as tc, ExitStack() as ctx:
        consts = ctx.enter_context(tc.tile_pool(name="consts", bufs=1))
        wpool = ctx.enter_context(tc.tile_pool(name="weights", bufs=1))
        xpool = ctx.enter_context(tc.tile_pool(name="x", bufs=2))
        qkpool = ctx.enter_context(tc.tile_pool(name="qk", bufs=16))
        qpad = ctx.enter_context(tc.tile_pool(name="qpad", bufs=1))
        vpool = ctx.enter_context(tc.tile_pool(name="v", bufs=8))
        ptpool = ctx.enter_context(tc.tile_pool(name="pt", bufs=8))
        rbpool = ctx.enter_context(tc.tile_pool(name="rb", bufs=2))
        otpool = ctx.enter_context(tc.tile_pool(name="ot", bufs=8))
        ypool = ctx.enter_context(tc.tile_pool(name="y", bufs=3))
        small = ctx.enter_context(tc.tile_pool(name="small", bufs=4))
        psA = ctx.enter_context(tc.tile_pool(name="psA", bufs=2, space="PSUM"))
        psS = ctx.enter_context(tc.tile_pool(name="psS", bufs=2, space="PSUM"))
        psO = ctx.enter_context(tc.tile_pool(name="psO", bufs=2, space="PSUM"))

        if True:
            bqk_sb = consts.tile([128, 16], FP32, tag="bqk")
            bout_sb = consts.tile([128, 8], FP32, tag="bout")
            dummy_sb = consts.tile([128, 512], BF, tag="dummy")

            # wqk_sb[c4][:, ct*512 + off]: weights for qk pair 2*c4+pp,
            # ct-major within the chunk; wv/wout are [128, ct*1024 + col]
            wqk_sb = [wpool.tile([128, 8 * 512], BF, tag=f"wqk{c4}", name="w")
                      for c4 in range(4)]
            wv_sb = wpool.tile([128, 8 * C], BF, tag="wv")
            wout_sb = wpool.tile([128, 8 * C], BF, tag="wout")

            def emit_w_qk_chunk(c4):
                nc.sync.dma_start(
                    out=wqk_sb[c4][:], in_=wqk[c4 * 128:(c4 + 1) * 128, :])

            def emit_w_v():
                nc.sync.dma_start(out=wv_sb[:], in_=wv[:, :])

            def emit_w_out():
                nc.sync.dma_start(out=wout_sb[:], in_=wout[:, :])

            # --- software-pipelined emission ---------------------------------
            # Dense matmul phases (QKV, proj) are interleaved into the
            # attention phase so the PE never idles (HAM stays at 2.4 GHz):
            #   A(0) | B(0)+C(0) with A(1) spread through | B(1)+C(1)
            x_sb = {}
            qk_sb = {}
            v_sb = {}
            oT_sb = {}

            def emit_x(seg):
                t = xpool.tile([128, 8 * M], BF, tag="x", name="x")
                nq = 4 if seg == 0 else 2
                step = 8 * M // nq
                for i in range(nq):
                    nc.sync.dma_start(
                        out=t[:, i * step:(i + 1) * step],
                        in_=xiT[seg * 128:(seg + 1) * 128, i * step:(i + 1) * step])
                x_sb[seg] = t

            QK_ORDER = [p + half for p in range(8) for half in (0, 8)]

            def emit_qkv_unit(seg, u):
                # units 0..15: qk e-tiles (interleaved q/k); 16..23: v halves
                if u < 16:
                    et = QK_ORDER[u]
                    p = et % 8
                    c4, off = p // 2, (p % 2) * 256 + (0 if et < 8 else 128)
                    ps = psA.tile([128, M], FP32, tag="psA", name="ps")
                    for ct in range(CT):
                        mm("qkv_qk",
                            ps[:],
                            lhsT=wqk_sb[c4][:, ct * 512 + off:ct * 512 + off + 128],
                            rhs=x_sb[seg][:, ct * M:(ct + 1) * M],
                            start=(ct == 0), stop=(ct == CT - 1))
                    if et < 8:
                        # q e-tile drains into the block-diagonal padded
                        # layout [128, 2, M]: block 0 = [qA; 0], block 1 =
                        # [0; qB] (zeros pre-memset once at startup), so
                        # scores run as FULL-width 128-partition matmuls
                        # (row-tiled 64-partition matmuls each exposed
                        # ~170ns of drain latency; full-array back-to-back
                        # matmuls hide it)
                        t = qpad_sb[(seg, et)]
                        halves = ((0, 0), (64, 1))
                        for hi, (row, blk) in enumerate(halves):
                            if (u + hi) % 2 == 0:
                                nc.scalar.activation(
                                    out=t[row:row + 64, blk, :],
                                    in_=ps[row:row + 64, :],
                                    func=mybir.ActivationFunctionType.Identity,
                                    bias=bqk_sb[row:row + 64, et:et + 1],
                                    scale=1.0)
                            else:
                                nc.vector.tensor_scalar_add(
                                    t[row:row + 64, blk, :],
                                    ps[row:row + 64, :],
                                    bqk_sb[row:row + 64, et:et + 1])
                    else:
                        t = qkpool.tile([128, M], BF, tag="qk", name="qk")
                        # psum drains alternate scalar/vector so neither
                        # engine saturates in the attention phases
                        if u % 2 == 0:
                            nc.scalar.activation(
                                out=t[:], in_=ps[:],
                                func=mybir.ActivationFunctionType.Identity,
                                bias=bqk_sb[:, et:et + 1], scale=1.0)
                        else:
                            nc.vector.tensor_scalar_add(
                                t[:], ps[:], bqk_sb[:, et:et + 1])
                        qk_sb.setdefault(seg, [None] * 16)[et] = t
                else:
                    tt, nf = divmod(u - 16, 2)
                    if nf == 0:
                        # per-head stationary [ones(64) | v(64)]: the PV
                        # matmul then emits 64 replicated denominator rows
                        # (partitions 0:64) alongside the 64 output rows
                        # (64:128), so normalization needs no partition
                        # broadcast and no [1,M] lane-bound DVE ops.
                        vt = vpool.tile([128, 16, 128], BF, tag="v", name="v")
                        v_sb.setdefault(seg, [None] * 4)[tt] = vt
                        nc.vector.memset(vt[:, :, 0:64], 1.0)
                    vt = v_sb[seg][tt]
                    ps = psA.tile([128, M], FP32, tag="psA", name="ps")
                    for ct in range(CT):
                        mm("qkv_v",
                            ps[:],
                            lhsT=x_sb[seg][:, ct * M + tt * 128:ct * M + (tt + 1) * 128],
                            rhs=wv_sb[:, ct * C + nf * 512:ct * C + (nf + 1) * 512],
                            start=(ct == 0), stop=(ct == CT - 1))
                    if u % 2 == 0:
                        nc.scalar.copy(
                            out=vt[:, nf * 8:(nf + 1) * 8, 64:128],
                            in_=ps[:].rearrange("p (h e) -> p h e", e=64))
                    else:
                        nc.vector.tensor_copy(
                            out=vt[:, nf * 8:(nf + 1) * 8, 64:128],
                            in_=ps[:].rearrange("p (h e) -> p h e", e=64))

            def emit_scores_half(seg, h, ps2, kc):
                # scoresT block [k, q] for one head as a FULL-width matmul:
                # lhsT is the k e-tile chunk (both heads stacked on 128
                # partitions); rhs is the padded q block [qA;0] or [0;qB],
                # whose zero rows cancel the other head's contribution.
                # ps2 is a 2-bank pair psum [128, 2, M]; half h%2 owns bank
                # h%2 so the pair's exp can run as ONE scalar instruction.
                et, blk = h // 2, h % 2
                n2 = (4 - kc) * 128
                q2 = qpad_sb[(seg, et)]
                kh = qk_sb[seg][8 + et]
                mm("scores",
                    ps2[:, blk, :n2],
                    lhsT=kh[:, kc * 128:(kc + 1) * 128],
                    rhs=q2[:, blk, kc * 128:], start=True, stop=True)

            def emit_scores_chunk(seg, p, kc):
                # One kc chunk for a head pair: two concurrent 64-row
                # matmuls into a 2-bank psum pair tile, one fused exp
                # (scalar), causal mask on gpsimd for the diagonal block.
                n2 = (4 - kc) * 128
                ps2 = psS.tile([128, 2, M], FP32, tag="psS", name="ps")
                emit_scores_half(seg, 2 * p, ps2, kc)
                emit_scores_half(seg, 2 * p + 1, ps2, kc)
                pt2 = ptpool.tile([128, 2, M], BF, tag="pt", name="pt")
                nc.scalar.activation(
                    out=pt2[:, :, :n2], in_=ps2[:, :, :n2],
                    func=mybir.ActivationFunctionType.Exp)
                for half in (0, 1):
                    nc.gpsimd.affine_select(
                        out=pt2[:, half, 0:128], in_=pt2[:, half, 0:128],
                        compare_op=mybir.AluOpType.is_ge,
                        fill=0.0, base=0,
                        pattern=[[1, 128]], channel_multiplier=-1)
                return pt2

            # scores for a pair are emitted in two halves with other PE work
            # in between: the kc2/kc3 psum tiles reuse kc0/kc1's banks (psS
            # has 2 pair-buffers), so kc2's matmul must not be emitted until
            # kc0's exp has had ~1us of PE stream to complete behind, or the
            # PE stalls on the bank WAR every pair.
            def emit_scores_front(seg, p):
                return [emit_scores_chunk(seg, p, 0),
                        emit_scores_chunk(seg, p, 1)]

            def emit_scores_back(seg, p, pts):
                pts.append(emit_scores_chunk(seg, p, 2))
                pts.append(emit_scores_chunk(seg, p, 3))
                return pts

            def emit_pv(seg, h, pts):
                # po [128, M]: rows 0:64 = denominator replicated 64x (from
                # the ones half of v), rows 64:128 = unnormalized outT
                half = h % 2
                po = psO.tile([128, M], FP32, tag="psO", name="po")
                for kc in range(4):
                    n2 = (4 - kc) * 128
                    mm("pv",
                        po[:, kc * 128:],
                        lhsT=v_sb[seg][kc][:, h, :],
                        rhs=pts[kc][:, half, :n2],
                        start=(kc == 0), stop=(kc == 3))
                rb = rbpool.tile([64, M], FP32, tag="rb", name="rb")
                nc.vector.reciprocal_approx_fast(out=rb[:], in_=po[0:64, :])
                nc.vector.tensor_mul(
                    out=oT_sb[seg][h // 2][half * 64:half * 64 + 64, :],
                    in0=po[64:128, :], in1=rb[:])

            def emit_pv_pair(seg, p, pts):
                emit_pv(seg, 2 * p, pts)
                emit_pv(seg, 2 * p + 1, pts)

            y2_sb = {}

            def emit_proj_tile(seg, ot):
                # output tiles pack in PAIRS [128, 2, 512] so the DMA rows
                # are 2KB contiguous (1KB bf16 rows halve DMA throughput and
                # were the ~6us kernel tail); host unpacks the pair layout
                b, j = divmod(ot, 2)
                ps = psA.tile([128, M], FP32, tag="psA", name="ps")
                for ct in range(CT):
                    mm("proj",
                        ps[:],
                        lhsT=wout_sb[:, ct * C + ot * 128:ct * C + (ot + 1) * 128],
                        rhs=oT_sb[seg][ct][:],
                        start=(ct == 0), stop=(ct == CT - 1))
                if j == 0:
                    y2_sb[(seg, b)] = ypool.tile([128, 2, M], BF, tag="y",
                                                 name="yt")
                yt = y2_sb[(seg, b)]
                if ot % 2 == 0:
                    nc.scalar.activation(
                        out=yt[:, j, :], in_=ps[:],
                        func=mybir.ActivationFunctionType.Identity,
                        bias=bout_sb[:, ot:ot + 1], scale=1.0)
                else:
                    nc.vector.tensor_scalar_add(
                        yt[:, j, :], ps[:], bout_sb[:, ot:ot + 1])
                if j == 1:
                    blk = seg * 4 + b
                    nc.sync.dma_start(
                        out=yT[blk * 128:(blk + 1) * 128, :], in_=yt[:])

            def emit_attn(seg, filler, warm):
                # two-stage software pipeline over head PAIRS: scores+exp of
                # pair p+1 are emitted before PV(p), covering softmax latency.
                # `warm` = pts for pair 0, pre-emitted into the preceding
                # dense stream.  Returns the final pair's pts; the CALLER
                # emits its PV (so the next segment's score front can slot
                # in between, keeping the psS bank-reuse slack).
                oT_sb[seg] = [otpool.tile([128, M], BF, tag="ot", name="ot")
                              for _ in range(8)]
                prev = None
                for p in range(H // 2):
                    if p == 0:
                        cur = (0, warm)
                    else:
                        front = emit_scores_front(seg, p)
                        if prev is not None:
                            emit_pv_pair(seg, prev[0], prev[1])
                            prev = None
                        cur = (p, emit_scores_back(seg, p, front))
                    if prev is not None:
                        emit_pv_pair(seg, prev[0], prev[1])
                    filler()
                    prev = cur
                return prev

            # --- PE warm-up: dummy matmuls during the initial DMA wait ----
            nc.vector.memset(dummy_sb[:], 0.125)
            wps = psA.tile([128, M], FP32, tag="psA", name="ps")
            for i in range(14):
                mm("warm", wps[:], lhsT=dummy_sb[:, 0:128], rhs=dummy_sb[:],
                   start=True, stop=True)

            # persistent padded-q tiles; zero halves written once, during
            # the startup DMA-wait window (vector is idle then)
            qpad_sb = {}
            for seg in range(2):
                for et in range(8):
                    t = qpad.tile([128, 2, M], BF, tag=f"qp{seg}_{et}",
                                  name="qp")
                    qpad_sb[(seg, et)] = t
                    nc.vector.memset(t[64:128, 0, :], 0.0)
                    nc.vector.memset(t[0:64, 1, :], 0.0)

            # --- startup DMAs: first wqk(0) quarter, then all of x(0)
            # (every unit streams all 8 x ct-chunks, so x gates the early
            # unit pipeline), then the remaining wqk(0) quarters.
            x0 = xpool.tile([128, 8 * M], BF, tag="x", name="x")
            x_sb[0] = x0
            for i in range(4):
                nc.sync.dma_start(
                    out=x0[:, i * 1024:(i + 1) * 1024],
                    in_=xiT[0:128, i * 1024:(i + 1) * 1024])
                nc.sync.dma_start(
                    out=wqk_sb[0][:, i * 1024:(i + 1) * 1024],
                    in_=wqk[0:128, i * 1024:(i + 1) * 1024])
            nc.sync.dma_start(out=bqk_sb[:], in_=bqk[:, :])
            nc.sync.dma_start(out=bout_sb[:], in_=bout[:, :])
            emit_w_qk_chunk(1)
            emit_w_v()
            emit_w_qk_chunk(2)
            emit_w_qk_chunk(3)
            emit_x(1)
            emit_w_out()
            for u in range(20):
                emit_qkv_unit(0, u)
            # warm pair 0 of seg0: scores spread into the QKV tail
            emit_qkv_unit(0, 20)
            emit_qkv_unit(0, 21)
            warm0_front = emit_scores_front(0, 0)
            emit_qkv_unit(0, 22)
            warm0_pts = emit_scores_back(0, 0, warm0_front)
            emit_qkv_unit(0, 23)

            # Dense filler, 2 units per attention iteration, uniformly over
            # BOTH segments' attention phases so the PE span per pair-iter
            # (~5us) always exceeds the worst non-PE engine (~4.3us) and the
            # HAM never demotes.  Deadlines: seg1 v units before attn(1)'s
            # first PV; seg1 qk pair p before scores_pair(1, p) at iter p-1;
            # proj0 only after attn(0) is fully done.
            #   attn(0) iters 0-3: v units (16..23); iters 4-7: qk pairs 0-3
            #   attn(1) iters 0-3: qk pairs 4-7;     iters 4-7: proj0 tiles
            fill0_units = [16, 17, 18, 19, 20, 21, 22, 23,
                           0, 1, 2, 3, 4, 5, 6, 7]
            fill1_units = [8, 9, 10, 11, 12, 13, 14, 15]
            fill0 = iter(fill0_units)
            fill1 = iter(fill1_units)
            proj0 = iter(range(8))

            def fill_a0():
                for _ in range(2):
                    u = next(fill0, None)
                    if u is not None:
                        emit_qkv_unit(1, u)

            def fill_a1():
                for _ in range(2):
                    u = next(fill1, None)
                    if u is not None:
                        emit_qkv_unit(1, u)
                        continue
                    ot = next(proj0, None)
                    if ot is not None:
                        emit_proj_tile(0, ot)

            last0 = emit_attn(0, fill_a0, warm=warm0_pts)
            # warm-start seg1's pipeline the same way: its first pair's
            # score front runs before seg0's final PV pair, the back after
            warm1_front = emit_scores_front(1, 0)
            emit_pv_pair(0, last0[0], last0[1])
            warm1_pts = emit_scores_back(1, 0, warm1_front)
            last1 = emit_attn(1, fill_a1, warm=warm1_pts)
            emit_pv_pair(1, last1[0], last1[1])
            for ot in range(8):
                emit_proj_tile(1, ot)

    nc.finalize()
    return nc


def _prep_inputs(x, w_in, b_in, w_out, b_out):
    x = np.asarray(x, dtype=np.float32)
    w_in = np.asarray(w_in, dtype=np.float32)
    b_in = np.asarray(b_in, dtype=np.float32)
    w_out = np.asarray(w_out, dtype=np.float32)
    b_out = np.asarray(b_out, dtype=np.float32)

    # fold 1/sqrt(dh) into the q rows of w_in / b_in
    w_in_s = w_in.copy()
    b_in_s = b_in.copy()
    w_in_s[:C] *= DH ** -0.5
    b_in_s[:C] *= DH ** -0.5

    w_inT0 = np.ascontiguousarray(w_in_s.T).astype(BF16)
    # permute qk columns into [q_p | k_p] pairs matching the consume order
    w_inT = w_inT0.copy()
    for p in range(8):
        w_inT[:, p * 256:p * 256 + 128] = w_inT0[:, p * 128:(p + 1) * 128]
        w_inT[:, p * 256 + 128:(p + 1) * 256] = \
            w_inT0[:, C + p * 128:C + (p + 1) * 128]
    # repack into ct-major chunk layouts (one DMA per chunk on device)
    wp = w_inT.reshape(8, 128, 3 * C)
    wqk = np.ascontiguousarray(np.concatenate(
        [wp[:, :, c4 * 512:(c4 + 1) * 512].transpose(1, 0, 2).reshape(128, 8 * 512)
         for c4 in range(4)], axis=0))                       # (512, 4096)
    wv = np.ascontiguousarray(
        wp[:, :, 2 * C:].transpose(1, 0, 2).reshape(128, 8 * C))  # (128, 8192)
    w_outT = np.ascontiguousarray(w_out.T).astype(BF16)
    wout = np.ascontiguousarray(
        w_outT.reshape(8, 128, C).transpose(1, 0, 2).reshape(128, 8 * C))
    bqk = np.ascontiguousarray(b_in_s[:2 * C].reshape(16, 128).T, dtype=np.float32)
    # v bias folds exactly into an effective output bias:
    #   (p @ (v + 1 b_v^T)) / denom = (p @ v)/denom + b_v
    b_out_eff = b_out + w_out @ b_in[2 * C:]
    bout = np.ascontiguousarray(b_out_eff.reshape(8, 128).T, dtype=np.float32)

    # dilated gather + transpose + ct-major pack: per-core (2*128, 8*M)
    xi = x.reshape(B, S, W_SEG, C)[:, :, ::RATE, :]        # (B, S, M, C)
    xiT = np.ascontiguousarray(xi.transpose(0, 1, 3, 2)).astype(BF16)  # (B,S,C,M)
    xiT = xiT.reshape(16, 8, 128, M).transpose(0, 2, 1, 3)  # (16,128,8,M)
    xiT = np.ascontiguousarray(xiT).reshape(N_CORES, SEG_PER_CORE * 128, 8 * M)


    in_maps = []
    for c in range(N_CORES):
        in_maps.append({
            "xiT": np.ascontiguousarray(xiT[c]),
            "wqk": wqk,
            "wv": wv,
            "wout": wout,
            "bqk": bqk,
            "bout": bout,
        })
    return in_maps


def kernel(x, w_in, b_in, w_out, b_out, _trace=False):
    if "nc" not in _CACHE:
        _CACHE["nc"] = _build()
    nc = _CACHE["nc"]

    in_maps = _prep_inputs(x, w_in, b_in, w_out, b_out)
    res = run_bass_kernel_spmd(
        nc, in_maps, core_ids=list(range(N_CORES)), trace=_trace)
    _CACHE["last_result"] = res

    out = np.zeros((B, N, C), dtype=np.float32)
    ov = out.reshape(B, S, W_SEG, C)
    for c in range(N_CORES):
        yTc = np.asarray(res.results[c]["yT"], dtype=np.float32)
        a = yTc.reshape(SEG_PER_CORE, 4, 128, 2, M)
        for seg in range(SEG_PER_CORE):
            gseg = c * SEG_PER_CORE + seg
            b, s = divmod(gseg, S)
            yseg = a[seg].transpose(0, 2, 1, 3).reshape(C, M)
            ov[b, s, ::RATE, :] = yseg.T
    return out


# revision 53
# speedup vs baseline: 1.0626x; 1.0626x over previous
# Dilated causal self-attention kernel for Trainium2 (8 NeuronCores).
#
# Reference computation (see problem):
#   x (4, 8192, 1024) -> reshape (4, 4, 2048, 1024) -> take every 4th token
#   -> per-segment causal MHA (16 heads, dh=64) -> scatter back into zeros.
#
# Sharding: 16 independent (batch, segment) attention problems, 2 per core.
# Host does the dilated gather + transpose + bf16 cast and the final scatter
# into the zero background; each core runs QKV -> per-head causal softmax
# attention -> output projection on its 2 segments.
#
# Device layout (all feature-major where possible):
#   xiT    [C, M]  (per segment)         - input, bf16
#   qkT    [2C, M] feature-major         - q rows pre-scaled by 1/sqrt(dh)
#                                          (folded into w_in on host)
#   v      [M, C]  token-major           - v bias folds into output bias
#   scores [128 q, n k] per (head, q-chunk), n = (qc+1)*128 (causal skip)
#   p = exp(scores) (no max subtraction: scores ~ N(0,1)), accum_out = denom
#   PV: outT[dh, M] = sum_kc v_kc^T @ pT_kc   (pT via PE transpose)
#   yT = w_outT^T @ oT + b_out_eff  -> DMA out feature-major (bf16)
#
# Perf changes vs the 206us baseline (measured 177us):
#   - scores emitted in head PAIRS: even head on PE row-group 0, odd head
#     on row-group 64 (K=dh=64 < 128) -> the two 64-row matmuls execute
#     concurrently (PE row tiling), halving score matmul time.
#   - per-kc fused exp: both heads' score chunks land in one 2-bank PSUM
#     pair tile [128,2,M], so one scalar activation covers the pair
#     (ACT per-instruction overhead ~290ns was saturating the scalar
#     engine in the attention tail and re-throttling the PE HAM).
#   - normalization without partition broadcasts: each head's PV
#     stationary is [ones(64) | v(64)], so the PV matmul itself emits 64
#     replicated denominator rows next to the 64 output rows; the whole
#     softmax divide is one [64,M] reciprocal + one [64,M] multiply on
#     the DVE (no gpsimd broadcast, no [1,M] single-lane DVE ops).
#   - dense filler (seg1 QKV, seg0 proj) spread uniformly, 2 units per
#     attention iteration across both segments' attention phases, so the
#     PE span per iteration (~5us) always exceeds the busiest non-PE
#     engine (~4us) and the HAM stays at K=8/8.
#   - startup: x(0)/wqk(0) quarter-DMAs interleaved so the first QKV
#     units' slices land first; dummy warm-up matmuls on a memset tile
#     run during the DMA-wait window to pre-warm the PE HAM state.
#   - causal mask via gpsimd affine_select on contiguous [128,128]
#     halves; psum drains alternate scalar/vector; output DMAs as bf16.

import sys

sys.path.insert(0, "/opt/trn_rl_repo")

import numpy as np
import ml_dtypes

import concourse.bacc as bacc
import concourse.mybir as mybir
from concourse.tile import TileContext
from concourse.bass_utils import run_bass_kernel_spmd

BF16 = ml_dtypes.bfloat16

B, N, C = 4, 8192, 1024
W_SEG, RATE, H = 2048, 4, 16
DH = C // H            # 64
S = N // W_SEG         # 4 segments per batch
M = W_SEG // RATE      # 512 tokens per segment
N_CORES = 8
SEG_PER_CORE = (B * S) // N_CORES  # 2

FP32 = mybir.dt.float32
BF = mybir.dt.bfloat16

_CACHE = {}


def _build():
    nc = bacc.Bacc()
    phase_of = _CACHE.setdefault("phase_of", {})

    def mm(phase, *args, **kwargs):
        inst = nc.tensor.matmul(*args, **kwargs)
        try:
            phase_of[inst.ins.name] = phase
        except Exception:
            pass
        return inst
    # chunk-major packed layouts (one DMA each; see _prep_inputs)
    xiT = nc.dram_tensor("xiT", [SEG_PER_CORE * 128, 8 * M], BF, kind="ExternalInput")
    wqk = nc.dram_tensor("wqk", [4 * 128, 8 * 512], BF, kind="ExternalInput")
    wv = nc.dram_tensor("wv", [128, 8 * C], BF, kind="ExternalInput")
    wout = nc.dram_tensor("wout", [128, 8 * C], BF, kind="ExternalInput")
    bqk = nc.dram_tensor("bqk", [128, 16], FP32, kind="ExternalInput")
    bout = nc.dram_tensor("bout", [128, 8], FP32, kind="ExternalInput")
    yT = nc.dram_tensor("yT", [SEG_PER_CORE * 4 * 128, 2 * M], BF,
                    kind="ExternalOutput")

    CT = C // 128  # 8 contraction chunks

    from contextlib import ExitStack
    with TileContext(nc) as tc, ExitStack() as ctx:
        consts = ctx.enter_context(tc.tile_pool(name="consts", bufs=1))
        wpool = ctx.enter_context(tc.tile_pool(name="weights", bufs=1))
        xpool = ctx.enter_context(tc.tile_pool(name="x", bufs=2))
        qkpool = ctx.enter_context(tc.tile_pool(name="qk", bufs=32))
        vpool = ctx.enter_context(tc.tile_pool(name="v", bufs=8))
        ptpool = ctx.enter_context(tc.tile_pool(name="pt", bufs=8))
        rbpool = ctx.enter_context(tc.tile_pool(name="rb", bufs=4))
        otpool = ctx.enter_context(tc.tile_pool(name="ot", bufs=8))
        ypool = ctx.enter_context(tc.tile_pool(name="y", bufs=3))
        small = ctx.enter_context(tc.tile_pool(name="small", bufs=4))
        psA = ctx.enter_context(tc.tile_pool(name="psA", bufs=2, space="PSUM"))
        psS = ctx.enter_context(tc.tile_pool(name="psS", bufs=2, space="PSUM"))
        psO = ctx.enter_context(tc.tile_pool(name="psO", bufs=2, space="PSUM"))

        if True:
            bqk_sb = consts.tile([128, 16], FP32, tag="bqk")
            bout_sb = consts.tile([128, 8], FP32, tag="bout")
            dummy_sb = consts.tile([128, 512], BF, tag="dummy")

            # wqk_sb[c4][:, ct*512 + off]: weights for qk pair 2*c4+pp,
            # ct-major within the chunk; wv/wout are [128, ct*1024 + col]
            wqk_sb = [wpool.tile([128, 8 * 512], BF, tag=f"wqk{c4}", name="w")
                      for c4 in range(4)]
            wv_sb = wpool.tile([128, 8 * C], BF, tag="wv")
            wout_sb = wpool.tile([128, 8 * C], BF, tag="wout")

            def emit_w_qk_chunk(c4):
                nc.sync.dma_start(
                    out=wqk_sb[c4][:], in_=wqk[c4 * 128:(c4 + 1) * 128, :])

            def emit_w_v():
                nc.sync.dma_start(out=wv_sb[:], in_=wv[:, :])

            def emit_w_out():
                nc.sync.dma_start(out=wout_sb[:], in_=wout[:, :])

            # --- software-pipelined emission ---------------------------------
            # Dense matmul phases (QKV, proj) are interleaved into the
            # attention phase so the PE never idles (HAM stays at 2.4 GHz):
            #   A(0) | B(0)+C(0) with A(1) spread through | B(1)+C(1)
            x_sb = {}
            qk_sb = {}
            v_sb = {}
            oT_sb = {}

            def emit_x(seg):
                t = xpool.tile([128, 8 * M], BF, tag="x", name="x")
                nq = 4 if seg == 0 else 2
                step = 8 * M // nq
                for i in range(nq):
                    nc.sync.dma_start(
                        out=t[:, i * step:(i + 1) * step],
                        in_=xiT[seg * 128:(seg + 1) * 128, i * step:(i + 1) * step])
                x_sb[seg] = t

            QK_ORDER = [p + half for p in range(8) for half in (0, 8)]

            def emit_qkv_unit(seg, u):
                # units 0..15: qk e-tiles (interleaved q/k); 16..23: v halves
                if u < 16:
                    et = QK_ORDER[u]
                    p = et % 8
                    c4, off = p // 2, (p % 2) * 256 + (0 if et < 8 else 128)
                    ps = psA.tile([128, M], FP32, tag="psA", name="ps")
                    for ct in range(CT):
                        mm("qkv_qk",
                            ps[:],
                            lhsT=wqk_sb[c4][:, ct * 512 + off:ct * 512 + off + 128],
                            rhs=x_sb[seg][:, ct * M:(ct + 1) * M],
                            start=(ct == 0), stop=(ct == CT - 1))
                    t = qkpool.tile([128, M], BF, tag="qk", name="qk")
                    # psum drains alternate scalar/vector so neither engine
                    # saturates when drains interleave with exps in the
                    # attention phases
                    if u % 2 == 0:
                        nc.scalar.activation(
                            out=t[:], in_=ps[:],
                            func=mybir.ActivationFunctionType.Identity,
                            bias=bqk_sb[:, et:et + 1], scale=1.0)
                    else:
                        nc.vector.tensor_scalar_add(
                            t[:], ps[:], bqk_sb[:, et:et + 1])
                    qk_sb.setdefault(seg, [None] * 16)[et] = t
                else:
                    tt, nf = divmod(u - 16, 2)
                    if nf == 0:
                        # per-head stationary [ones(64) | v(64)]: the PV
                        # matmul then emits 64 replicated denominator rows
                        # (partitions 0:64) alongside the 64 output rows
                        # (64:128), so normalization needs no partition
                        # broadcast and no [1,M] lane-bound DVE ops.
                        vt = vpool.tile([128, 16, 128], BF, tag="v", name="v")
                        v_sb.setdefault(seg, [None] * 4)[tt] = vt
                        nc.vector.memset(vt[:, :, 0:64], 1.0)
                    vt = v_sb[seg][tt]
                    ps = psA.tile([128, M], FP32, tag="psA", name="ps")
                    for ct in range(CT):
                        mm("qkv_v",
                            ps[:],
                            lhsT=x_sb[seg][:, ct * M + tt * 128:ct * M + (tt + 1) * 128],
                            rhs=wv_sb[:, ct * C + nf * 512:ct * C + (nf + 1) * 512],
                            start=(ct == 0), stop=(ct == CT - 1))
                    if u % 2 == 0:
                        nc.scalar.copy(
                            out=vt[:, nf * 8:(nf + 1) * 8, 64:128],
                            in_=ps[:].rearrange("p (h e) -> p h e", e=64))
                    else:
                        nc.vector.tensor_copy(
                            out=vt[:, nf * 8:(nf + 1) * 8, 64:128],
                            in_=ps[:].rearrange("p (h e) -> p h e", e=64))

            def emit_scores_half(seg, h, ps2, kc):
                # scoresT block [k, q] for one head: lhsT/rhs live on
                # partitions (h%2)*64..+64, so even/odd heads land on PE
                # row-groups 0/64 and execute concurrently (row tiling).
                # ps2 is a 2-bank pair psum [128, 2, M]; half h%2 owns bank
                # h%2 so the pair's exp can run as ONE scalar instruction.
                et, row = h // 2, (h % 2) * 64
                n2 = (4 - kc) * 128
                qh = qk_sb[seg][et][row:row + 64, :]
                kh = qk_sb[seg][8 + et][row:row + 64, :]
                mm("scores",
                    ps2[:, h % 2, :n2],
                    lhsT=kh[:, kc * 128:(kc + 1) * 128],
                    rhs=qh[:, kc * 128:], start=True, stop=True)

            def emit_scores_chunk(seg, p, kc):
                # One kc chunk for a head pair: two concurrent 64-row
                # matmuls into a 2-bank psum pair tile, one fused exp
                # (scalar), causal mask on gpsimd for the diagonal block.
                n2 = (4 - kc) * 128
                ps2 = psS.tile([128, 2, M], FP32, tag="psS", name="ps")
                emit_scores_half(seg, 2 * p, ps2, kc)
                emit_scores_half(seg, 2 * p + 1, ps2, kc)
                pt2 = ptpool.tile([128, 2, M], BF, tag="pt", name="pt")
                nc.scalar.activation(
                    out=pt2[:, :, :n2], in_=ps2[:, :, :n2],
                    func=mybir.ActivationFunctionType.Exp)
                for half in (0, 1):
                    nc.gpsimd.affine_select(
                        out=pt2[:, half, 0:128], in_=pt2[:, half, 0:128],
                        compare_op=mybir.AluOpType.is_ge,
                        fill=0.0, base=0,
                        pattern=[[1, 128]], channel_multiplier=-1)
                return pt2

            # scores for a pair are emitted in two halves with other PE work
            # in between: the kc2/kc3 psum tiles reuse kc0/kc1's banks (psS
            # has 2 pair-buffers), so kc2's matmul must not be emitted until
            # kc0's exp has had ~1us of PE stream to complete behind, or the
            # PE stalls on the bank WAR every pair.
            def emit_scores_front(seg, p):
                return [emit_scores_chunk(seg, p, 0),
                        emit_scores_chunk(seg, p, 1)]

            def emit_scores_back(seg, p, pts):
                pts.append(emit_scores_chunk(seg, p, 2))
                pts.append(emit_scores_chunk(seg, p, 3))
                return pts

            def emit_pv(seg, h, pts):
                # po [128, M]: rows 0:64 = denominator replicated 64x (from
                # the ones half of v), rows 64:128 = unnormalized outT
                half = h % 2
                po = psO.tile([128, M], FP32, tag="psO", name="po")
                for kc in range(4):
                    n2 = (4 - kc) * 128
                    mm("pv",
                        po[:, kc * 128:],
                        lhsT=v_sb[seg][kc][:, h, :],
                        rhs=pts[kc][:, half, :n2],
                        start=(kc == 0), stop=(kc == 3))
                rb = rbpool.tile([64, M], FP32, tag="rb", name="rb")
                nc.vector.reciprocal_approx_fast(out=rb[:], in_=po[0:64, :])
                nc.vector.tensor_mul(
                    out=oT_sb[seg][h // 2][half * 64:half * 64 + 64, :],
                    in0=po[64:128, :], in1=rb[:])

            def emit_pv_pair(seg, p, pts):
                emit_pv(seg, 2 * p, pts)
                emit_pv(seg, 2 * p + 1, pts)

            y2_sb = {}

            def emit_proj_tile(seg, ot):
                # output tiles pack in PAIRS [128, 2, 512] so the DMA rows
                # are 2KB contiguous (1KB bf16 rows halve DMA throughput and
                # were the ~6us kernel tail); host unpacks the pair layout
                b, j = divmod(ot, 2)
                ps = psA.tile([128, M], FP32, tag="psA", name="ps")
                for ct in range(CT):
                    mm("proj",
                        ps[:],
                        lhsT=wout_sb[:, ct * C + ot * 128:ct * C + (ot + 1) * 128],
                        rhs=oT_sb[seg][ct][:],
                        start=(ct == 0), stop=(ct == CT - 1))
                if j == 0:
                    y2_sb[(seg, b)] = ypool.tile([128, 2, M], BF, tag="y",
                                                 name="yt")
                yt = y2_sb[(seg, b)]
                if ot % 2 == 0:
                    nc.scalar.activation(
                        out=yt[:, j, :], in_=ps[:],
                        func=mybir.ActivationFunctionType.Identity,
                        bias=bout_sb[:, ot:ot + 1], scale=1.0)
                else:
                    nc.vector.tensor_scalar_add(
                        yt[:, j, :], ps[:], bout_sb[:, ot:ot + 1])
                if j == 1:
                    blk = seg * 4 + b
                    nc.sync.dma_start(
                        out=yT[blk * 128:(blk + 1) * 128, :], in_=yt[:])

            def emit_attn(seg, filler, warm):
                # two-stage software pipeline over head PAIRS: scores+exp of
                # pair p+1 are emitted before PV(p), covering softmax latency.
                # `warm` = pts for pair 0, pre-emitted into the preceding
                # dense stream.  Returns the final pair's pts; the CALLER
                # emits its PV (so the next segment's score front can slot
                # in between, keeping the psS bank-reuse slack).
                oT_sb[seg] = [otpool.tile([128, M], BF, tag="ot", name="ot")
                              for _ in range(8)]
                prev = None
                for p in range(H // 2):
                    if p == 0:
                        cur = (0, warm)
                    else:
                        front = emit_scores_front(seg, p)
                        if prev is not None:
                            emit_pv_pair(seg, prev[0], prev[1])
                            prev = None
                        cur = (p, emit_scores_back(seg, p, front))
                    if prev is not None:
                        emit_pv_pair(seg, prev[0], prev[1])
                    filler()
                    prev = cur
                return prev

            # --- PE warm-up: dummy matmuls during the initial DMA wait ----
            nc.vector.memset(dummy_sb[:], 0.125)
            wps = psA.tile([128, M], FP32, tag="psA", name="ps")
            for i in range(14):
                mm("warm", wps[:], lhsT=dummy_sb[:, 0:128], rhs=dummy_sb[:],
                   start=True, stop=True)

            # --- startup DMAs: first wqk(0) quarter, then all of x(0)
            # (every unit streams all 8 x ct-chunks, so x gates the early
            # unit pipeline), then the remaining wqk(0) quarters.
            x0 = xpool.tile([128, 8 * M], BF, tag="x", name="x")
            x_sb[0] = x0
            for i in range(4):
                nc.sync.dma_start(
                    out=x0[:, i * 1024:(i + 1) * 1024],
                    in_=xiT[0:128, i * 1024:(i + 1) * 1024])
                nc.sync.dma_start(
                    out=wqk_sb[0][:, i * 1024:(i + 1) * 1024],
                    in_=wqk[0:128, i * 1024:(i + 1) * 1024])
            nc.sync.dma_start(out=bqk_sb[:], in_=bqk[:, :])
            nc.sync.dma_start(out=bout_sb[:], in_=bout[:, :])
            emit_w_qk_chunk(1)
            emit_w_v()
            emit_w_qk_chunk(2)
            emit_w_qk_chunk(3)
            emit_x(1)
            emit_w_out()
            for u in range(20):
                emit_qkv_unit(0, u)
            # warm pair 0 of seg0: scores spread into the QKV tail
            emit_qkv_unit(0, 20)
            emit_qkv_unit(0, 21)
            warm0_front = emit_scores_front(0, 0)
            emit_qkv_unit(0, 22)
            warm0_pts = emit_scores_back(0, 0, warm0_front)
            emit_qkv_unit(0, 23)

            # Dense filler, 2 units per attention iteration, uniformly over
            # BOTH segments' attention phases so the PE span per pair-iter
            # (~5us) always exceeds the worst non-PE engine (~4.3us) and the
            # HAM never demotes.  Deadlines: seg1 v units before attn(1)'s
            # first PV; seg1 qk pair p before scores_pair(1, p) at iter p-1;
            # proj0 only after attn(0) is fully done.
            #   attn(0) iters 0-3: v units (16..23); iters 4-7: qk pairs 0-3
            #   attn(1) iters 0-3: qk pairs 4-7;     iters 4-7: proj0 tiles
            fill0_units = [16, 17, 18, 19, 20, 21, 22, 23,
                           0, 1, 2, 3, 4, 5, 6, 7]
            fill1_units = [8, 9, 10, 11, 12, 13, 14, 15]
            fill0 = iter(fill0_units)
            fill1 = iter(fill1_units)
            proj0 = iter(range(8))

            def fill_a0():
                for _ in range(2):
                    u = next(fill0, None)
                    if u is not None:
                        emit_qkv_unit(1, u)

            def fill_a1():
                for _ in range(2):
                    u = next(fill1, None)
                    if u is not None:
                        emit_qkv_unit(1, u)
                        continue
                    ot = next(proj0, None)
                    if ot is not None:
                        emit_proj_tile(0, ot)

            last0 = emit_attn(0, fill_a0, warm=warm0_pts)
            # warm-start seg1's pipeline the same way: its first pair's
            # score front runs before seg0's final PV pair, the back after
            warm1_front = emit_scores_front(1, 0)
            emit_pv_pair(0, last0[0], last0[1])
            warm1_pts = emit_scores_back(1, 0, warm1_front)
            last1 = emit_attn(1, fill_a1, warm=warm1_pts)
            emit_pv_pair(1, last1[0], last1[1])
            for ot in range(8):
                emit_proj_tile(1, ot)

    nc.finalize()
    return nc


def _prep_inputs(x, w_in, b_in, w_out, b_out):
    x = np.asarray(x, dtype=np.float32)
    w_in = np.asarray(w_in, dtype=np.float32)
    b_in = np.asarray(b_in, dtype=np.float32)
    w_out = np.asarray(w_out, dtype=np.float32)
    b_out = np.asarray(b_out, dtype=np.float32)

    # fold 1/sqrt(dh) into the q rows of w_in / b_in
    w_in_s = w_in.copy()
    b_in_s = b_in.copy()
    w_in_s[:C] *= DH ** -0.5
    b_in_s[:C] *= DH ** -0.5

    w_inT0 = np.ascontiguousarray(w_in_s.T).astype(BF16)
    # permute qk columns into [q_p | k_p] pairs matching the consume order
    w_inT = w_inT0.copy()
    for p in range(8):
        w_inT[:, p * 256:p * 256 + 128] = w_inT0[:, p * 128:(p + 1) * 128]
        w_inT[:, p * 256 + 128:(p + 1) * 256] = \
            w_inT0[:, C + p * 128:C + (p + 1) * 128]
    # repack into ct-major chunk layouts (one DMA per chunk on device)
    wp = w_inT.reshape(8, 128, 3 * C)
    wqk = np.ascontiguousarray(np.concatenate(
        [wp[:, :, c4 * 512:(c4 + 1) * 512].transpose(1, 0, 2).reshape(128, 8 * 512)
         for c4 in range(4)], axis=0))                       # (512, 4096)
    wv = np.ascontiguousarray(
        wp[:, :, 2 * C:].transpose(1, 0, 2).reshape(128, 8 * C))  # (128, 8192)
    w_outT = np.ascontiguousarray(w_out.T).astype(BF16)
    wout = np.ascontiguousarray(
        w_outT.reshape(8, 128, C).transpose(1, 0, 2).reshape(128, 8 * C))
    bqk = np.ascontiguousarray(b_in_s[:2 * C].reshape(16, 128).T, dtype=np.float32)
    # v bias folds exactly into an effective output bias:
    #   (p @ (v + 1 b_v^T)) / denom = (p @ v)/denom + b_v
    b_out_eff = b_out + w_out @ b_in[2 * C:]
    bout = np.ascontiguousarray(b_out_eff.reshape(8, 128).T, dtype=np.float32)

    # dilated gather + transpose + ct-major pack: per-core (2*128, 8*M)
    xi = x.reshape(B, S, W_SEG, C)[:, :, ::RATE, :]        # (B, S, M, C)
    xiT = np.ascontiguousarray(xi.transpose(0, 1, 3, 2)).astype(BF16)  # (B,S,C,M)
    xiT = xiT.reshape(16, 8, 128, M).transpose(0, 2, 1, 3)  # (16,128,8,M)
    xiT = np.ascontiguousarray(xiT).reshape(N_CORES, SEG_PER_CORE * 128, 8 * M)


    in_maps = []
    for c in range(N_CORES):
        in_maps.append({
            "xiT": np.ascontiguousarray(xiT[c]),
            "wqk": wqk,
            "wv": wv,
            "wout": wout,
            "bqk": bqk,
            "bout": bout,
        })
    return in_maps


def kernel(x, w_in, b_in, w_out, b_out, _trace=False):
    if "nc" not in _CACHE:
        _CACHE["nc"] = _build()
    nc = _CACHE["nc"]

    in_maps = _prep_inputs(x, w_in, b_in, w_out, b_out)
    res = run_bass_kernel_spmd(
        nc, in_maps, core_ids=list(range(N_CORES)), trace=_trace)
    _CACHE["last_result"] = res

    out = np.zeros((B, N, C), dtype=np.float32)
    ov = out.reshape(B, S, W_SEG, C)
    for c in range(N_CORES):
        yTc = np.asarray(res.results[c]["yT"], dtype=np.float32)
        a = yTc.reshape(SEG_PER_CORE, 4, 128, 2, M)
        for seg in range(SEG_PER_CORE):
            gseg = c * SEG_PER_CORE + seg
            b, s = divmod(gseg, S)
            yseg = a[seg].transpose(0, 2, 1, 3).reshape(C, M)
            ov[b, s, ::RATE, :] = yseg.T
    return out


# revision 54
# speedup vs baseline: 1.0656x; 1.0027x over previous
# Dilated causal self-attention kernel for Trainium2 (8 NeuronCores).
#
# Reference computation (see problem):
#   x (4, 8192, 1024) -> reshape (4, 4, 2048, 1024) -> take every 4th token
#   -> per-segment causal MHA (16 heads, dh=64) -> scatter back into zeros.
#
# Sharding: 16 independent (batch, segment) attention problems, 2 per core.
# Host does the dilated gather + transpose + bf16 cast and the final scatter
# into the zero background; each core runs QKV -> per-head causal softmax
# attention -> output projection on its 2 segments.
#
# Device layout (all feature-major where possible):
#   xiT    [C, M]  (per segment)         - input, bf16
#   qkT    [2C, M] feature-major         - q rows pre-scaled by 1/sqrt(dh)
#                                          (folded into w_in on host)
#   v      [M, C]  token-major           - v bias folds into output bias
#   scores [128 q, n k] per (head, q-chunk), n = (qc+1)*128 (causal skip)
#   p = exp(scores) (no max subtraction: scores ~ N(0,1)), accum_out = denom
#   PV: outT[dh, M] = sum_kc v_kc^T @ pT_kc   (pT via PE transpose)
#   yT = w_outT^T @ oT + b_out_eff  -> DMA out feature-major (bf16)
#
# Perf changes vs the 206us baseline (measured 177us):
#   - scores emitted in head PAIRS: even head on PE row-group 0, odd head
#     on row-group 64 (K=dh=64 < 128) -> the two 64-row matmuls execute
#     concurrently (PE row tiling), halving score matmul time.
#   - per-kc fused exp: both heads' score chunks land in one 2-bank PSUM
#     pair tile [128,2,M], so one scalar activation covers the pair
#     (ACT per-instruction overhead ~290ns was saturating the scalar
#     engine in the attention tail and re-throttling the PE HAM).
#   - normalization without partition broadcasts: each head's PV
#     stationary is [ones(64) | v(64)], so the PV matmul itself emits 64
#     replicated denominator rows next to the 64 output rows; the whole
#     softmax divide is one [64,M] reciprocal + one [64,M] multiply on
#     the DVE (no gpsimd broadcast, no [1,M] single-lane DVE ops).
#   - dense filler (seg1 QKV, seg0 proj) spread uniformly, 2 units per
#     attention iteration across both segments' attention phases, so the
#     PE span per iteration (~5us) always exceeds the busiest non-PE
#     engine (~4us) and the HAM stays at K=8/8.
#   - startup: x(0)/wqk(0) quarter-DMAs interleaved so the first QKV
#     units' slices land first; dummy warm-up matmuls on a memset tile
#     run during the DMA-wait window to pre-warm the PE HAM state.
#   - causal mask via gpsimd affine_select on contiguous [128,128]
#     halves; psum drains alternate scalar/vector; output DMAs as bf16.

import sys

sys.path.insert(0, "/opt/trn_rl_repo")

import numpy as np
import ml_dtypes

import concourse.bacc as bacc
import concourse.mybir as mybir
from concourse.tile import TileContext
from concourse.bass_utils import run_bass_kernel_spmd

BF16 = ml_dtypes.bfloat16

B, N, C = 4, 8192, 1024
W_SEG, RATE, H = 2048, 4, 16
DH = C // H            # 64
S = N // W_SEG         # 4 segments per batch
M = W_SEG // RATE      # 512 tokens per segment
N_CORES = 8
SEG_PER_CORE = (B * S) // N_CORES  # 2

FP32 = mybir.dt.float32
BF = mybir.dt.bfloat16

_CACHE = {}


def _build():
    nc = bacc.Bacc()
    phase_of = _CACHE.setdefault("phase_of", {})

    def mm(phase, *args, **kwargs):
        inst = nc.tensor.matmul(*args, **kwargs)
        try:
            phase_of[inst.ins.name] = phase
        except Exception:
            pass
        return inst
    # chunk-major packed layouts (one DMA each; see _prep_inputs)
    xiT = nc.dram_tensor("xiT", [SEG_PER_CORE * 128, 8 * M], BF, kind="ExternalInput")
    wqk = nc.dram_tensor("wqk", [4 * 128, 8 * 512], BF, kind="ExternalInput")
    wv = nc.dram_tensor("wv", [128, 8 * C], BF, kind="ExternalInput")
    wout = nc.dram_tensor("wout", [128, 8 * C], BF, kind="ExternalInput")
    bqk = nc.dram_tensor("bqk", [128, 16], FP32, kind="ExternalInput")
    bout = nc.dram_tensor("bout", [128, 8], FP32, kind="ExternalInput")
    yT = nc.dram_tensor("yT", [SEG_PER_CORE * 4 * 128, 2 * M], BF,
                    kind="ExternalOutput")

    CT = C // 128  # 8 contraction chunks

    from contextlib import ExitStack
    with TileContext(nc) as tc, ExitStack() as ctx:
        consts = ctx.enter_context(tc.tile_pool(name="consts", bufs=1))
        wpool = ctx.enter_context(tc.tile_pool(name="weights", bufs=1))
        xpool = ctx.enter_context(tc.tile_pool(name="x", bufs=2))
        qkpool = ctx.enter_context(tc.tile_pool(name="qk", bufs=32))
        vpool = ctx.enter_context(tc.tile_pool(name="v", bufs=8))
        ptpool = ctx.enter_context(tc.tile_pool(name="pt", bufs=8))
        rbpool = ctx.enter_context(tc.tile_pool(name="rb", bufs=4))
        otpool = ctx.enter_context(tc.tile_pool(name="ot", bufs=8))
        ypool = ctx.enter_context(tc.tile_pool(name="y", bufs=3))
        small = ctx.enter_context(tc.tile_pool(name="small", bufs=4))
        psA = ctx.enter_context(tc.tile_pool(name="psA", bufs=2, space="PSUM"))
        psS = ctx.enter_context(tc.tile_pool(name="psS", bufs=2, space="PSUM"))
        psO = ctx.enter_context(tc.tile_pool(name="psO", bufs=2, space="PSUM"))

        if True:
            bqk_sb = consts.tile([128, 16], FP32, tag="bqk")
            bout_sb = consts.tile([128, 8], FP32, tag="bout")
            dummy_sb = consts.tile([128, 512], BF, tag="dummy")

            # wqk_sb[c4][:, ct*512 + off]: weights for qk pair 2*c4+pp,
            # ct-major within the chunk; wv/wout are [128, ct*1024 + col]
            wqk_sb = [wpool.tile([128, 8 * 512], BF, tag=f"wqk{c4}", name="w")
                      for c4 in range(4)]
            wv_sb = wpool.tile([128, 8 * C], BF, tag="wv")
            wout_sb = wpool.tile([128, 8 * C], BF, tag="wout")

            def emit_w_qk_chunk(c4):
                nc.sync.dma_start(
                    out=wqk_sb[c4][:], in_=wqk[c4 * 128:(c4 + 1) * 128, :])

            def emit_w_v():
                nc.sync.dma_start(out=wv_sb[:], in_=wv[:, :])

            def emit_w_out():
                nc.sync.dma_start(out=wout_sb[:], in_=wout[:, :])

            # --- software-pipelined emission ---------------------------------
            # Dense matmul phases (QKV, proj) are interleaved into the
            # attention phase so the PE never idles (HAM stays at 2.4 GHz):
            #   A(0) | B(0)+C(0) with A(1) spread through | B(1)+C(1)
            x_sb = {}
            qk_sb = {}
            v_sb = {}
            oT_sb = {}

            def emit_x(seg):
                t = xpool.tile([128, 8 * M], BF, tag="x", name="x")
                nq = 4 if seg == 0 else 2
                step = 8 * M // nq
                for i in range(nq):
                    nc.sync.dma_start(
                        out=t[:, i * step:(i + 1) * step],
                        in_=xiT[seg * 128:(seg + 1) * 128, i * step:(i + 1) * step])
                x_sb[seg] = t

            QK_ORDER = [p + half for p in range(8) for half in (0, 8)]

            def emit_qkv_unit(seg, u):
                # units 0..15: qk e-tiles (interleaved q/k); 16..23: v halves
                if u < 16:
                    et = QK_ORDER[u]
                    p = et % 8
                    c4, off = p // 2, (p % 2) * 256 + (0 if et < 8 else 128)
                    ps = psA.tile([128, M], FP32, tag="psA", name="ps")
                    for ct in range(CT):
                        mm("qkv_qk",
                            ps[:],
                            lhsT=wqk_sb[c4][:, ct * 512 + off:ct * 512 + off + 128],
                            rhs=x_sb[seg][:, ct * M:(ct + 1) * M],
                            start=(ct == 0), stop=(ct == CT - 1))
                    t = qkpool.tile([128, M], BF, tag="qk", name="qk")
                    # psum drains alternate scalar/vector so neither engine
                    # saturates when drains interleave with exps in the
                    # attention phases
                    if u % 2 == 0:
                        nc.scalar.activation(
                            out=t[:], in_=ps[:],
                            func=mybir.ActivationFunctionType.Identity,
                            bias=bqk_sb[:, et:et + 1], scale=1.0)
                    else:
                        nc.vector.tensor_scalar_add(
                            t[:], ps[:], bqk_sb[:, et:et + 1])
                    qk_sb.setdefault(seg, [None] * 16)[et] = t
                else:
                    tt, nf = divmod(u - 16, 2)
                    if nf == 0:
                        # per-head stationary [ones(64) | v(64)]: the PV
                        # matmul then emits 64 replicated denominator rows
                        # (partitions 0:64) alongside the 64 output rows
                        # (64:128), so normalization needs no partition
                        # broadcast and no [1,M] lane-bound DVE ops.
                        vt = vpool.tile([128, 16, 128], BF, tag="v", name="v")
                        v_sb.setdefault(seg, [None] * 4)[tt] = vt
                        nc.vector.memset(vt[:, :, 0:64], 1.0)
                    vt = v_sb[seg][tt]
                    ps = psA.tile([128, M], FP32, tag="psA", name="ps")
                    for ct in range(CT):
                        mm("qkv_v",
                            ps[:],
                            lhsT=x_sb[seg][:, ct * M + tt * 128:ct * M + (tt + 1) * 128],
                            rhs=wv_sb[:, ct * C + nf * 512:ct * C + (nf + 1) * 512],
                            start=(ct == 0), stop=(ct == CT - 1))
                    if u % 2 == 0:
                        nc.scalar.copy(
                            out=vt[:, nf * 8:(nf + 1) * 8, 64:128],
                            in_=ps[:].rearrange("p (h e) -> p h e", e=64))
                    else:
                        nc.vector.tensor_copy(
                            out=vt[:, nf * 8:(nf + 1) * 8, 64:128],
                            in_=ps[:].rearrange("p (h e) -> p h e", e=64))

            def emit_scores_half(seg, h, ps2, kc):
                # scoresT block [k, q] for one head: lhsT/rhs live on
                # partitions (h%2)*64..+64, so even/odd heads land on PE
                # row-groups 0/64 and execute concurrently (row tiling).
                # ps2 is a 2-bank pair psum [128, 2, M]; half h%2 owns bank
                # h%2 so the pair's exp can run as ONE scalar instruction.
                et, row = h // 2, (h % 2) * 64
                n2 = (4 - kc) * 128
                qh = qk_sb[seg][et][row:row + 64, :]
                kh = qk_sb[seg][8 + et][row:row + 64, :]
                mm("scores",
                    ps2[:, h % 2, :n2],
                    lhsT=kh[:, kc * 128:(kc + 1) * 128],
                    rhs=qh[:, kc * 128:], start=True, stop=True)

            def emit_scores_chunk(seg, p, kc):
                # One kc chunk for a head pair: two concurrent 64-row
                # matmuls into a 2-bank psum pair tile, one fused exp
                # (scalar), causal mask on gpsimd for the diagonal block.
                n2 = (4 - kc) * 128
                ps2 = psS.tile([128, 2, M], FP32, tag="psS", name="ps")
                emit_scores_half(seg, 2 * p, ps2, kc)
                emit_scores_half(seg, 2 * p + 1, ps2, kc)
                pt2 = ptpool.tile([128, 2, M], BF, tag="pt", name="pt")
                nc.scalar.activation(
                    out=pt2[:, :, :n2], in_=ps2[:, :, :n2],
                    func=mybir.ActivationFunctionType.Exp)
                for half in (0, 1):
                    nc.gpsimd.affine_select(
                        out=pt2[:, half, 0:128], in_=pt2[:, half, 0:128],
                        compare_op=mybir.AluOpType.is_ge,
                        fill=0.0, base=0,
                        pattern=[[1, 128]], channel_multiplier=-1)
                return pt2

            # scores for a pair are emitted in two halves with other PE work
            # in between: the kc2/kc3 psum tiles reuse kc0/kc1's banks (psS
            # has 2 pair-buffers), so kc2's matmul must not be emitted until
            # kc0's exp has had ~1us of PE stream to complete behind, or the
            # PE stalls on the bank WAR every pair.
            def emit_scores_front(seg, p):
                return [emit_scores_chunk(seg, p, 0),
                        emit_scores_chunk(seg, p, 1)]

            def emit_scores_back(seg, p, pts):
                pts.append(emit_scores_chunk(seg, p, 2))
                pts.append(emit_scores_chunk(seg, p, 3))
                return pts

            def emit_pv(seg, h, pts):
                # po [128, M]: rows 0:64 = denominator replicated 64x (from
                # the ones half of v), rows 64:128 = unnormalized outT
                half = h % 2
                po = psO.tile([128, M], FP32, tag="psO", name="po")
                for kc in range(4):
                    n2 = (4 - kc) * 128
                    mm("pv",
                        po[:, kc * 128:],
                        lhsT=v_sb[seg][kc][:, h, :],
                        rhs=pts[kc][:, half, :n2],
                        start=(kc == 0), stop=(kc == 3))
                rb = rbpool.tile([64, M], FP32, tag="rb", name="rb")
                nc.vector.reciprocal_approx_fast(out=rb[:], in_=po[0:64, :])
                nc.vector.tensor_mul(
                    out=oT_sb[seg][h // 2][half * 64:half * 64 + 64, :],
                    in0=po[64:128, :], in1=rb[:])

            def emit_pv_pair(seg, p, pts):
                emit_pv(seg, 2 * p, pts)
                emit_pv(seg, 2 * p + 1, pts)

            y2_sb = {}

            def emit_proj_tile(seg, ot):
                # output tiles pack in PAIRS [128, 2, 512] so the DMA rows
                # are 2KB contiguous (1KB bf16 rows halve DMA throughput and
                # were the ~6us kernel tail); host unpacks the pair layout
                b, j = divmod(ot, 2)
                ps = psA.tile([128, M], FP32, tag="psA", name="ps")
                for ct in range(CT):
                    mm("proj",
                        ps[:],
                        lhsT=wout_sb[:, ct * C + ot * 128:ct * C + (ot + 1) * 128],
                        rhs=oT_sb[seg][ct][:],
                        start=(ct == 0), stop=(ct == CT - 1))
                if j == 0:
                    y2_sb[(seg, b)] = ypool.tile([128, 2, M], BF, tag="y",
                                                 name="yt")
                yt = y2_sb[(seg, b)]
                if ot % 2 == 0:
                    nc.scalar.activation(
                        out=yt[:, j, :], in_=ps[:],
                        func=mybir.ActivationFunctionType.Identity,
                        bias=bout_sb[:, ot:ot + 1], scale=1.0)
                else:
                    nc.vector.tensor_scalar_add(
                        yt[:, j, :], ps[:], bout_sb[:, ot:ot + 1])
                if j == 1:
                    blk = seg * 4 + b
                    nc.sync.dma_start(
                        out=yT[blk * 128:(blk + 1) * 128, :], in_=yt[:])

            def emit_attn(seg, filler, warm):
                # two-stage software pipeline over head PAIRS: scores+exp of
                # pair p+1 are emitted before PV(p), covering softmax latency.
                # `warm` = pts for pair 0, pre-emitted into the preceding
                # dense stream.  Returns the final pair's pts; the CALLER
                # emits its PV (so the next segment's score front can slot
                # in between, keeping the psS bank-reuse slack).
                oT_sb[seg] = [otpool.tile([128, M], BF, tag="ot", name="ot")
                              for _ in range(8)]
                prev = None
                for p in range(H // 2):
                    if p == 0:
                        cur = (0, warm)
                    else:
                        front = emit_scores_front(seg, p)
                        if prev is not None:
                            emit_pv_pair(seg, prev[0], prev[1])
                            prev = None
                        cur = (p, emit_scores_back(seg, p, front))
                    if prev is not None:
                        emit_pv_pair(seg, prev[0], prev[1])
                    filler()
                    prev = cur
                return prev

            # --- PE warm-up: dummy matmuls during the initial DMA wait ----
            nc.vector.memset(dummy_sb[:], 0.125)
            wps = psA.tile([128, M], FP32, tag="psA", name="ps")
            for i in range(14):
                mm("warm", wps[:], lhsT=dummy_sb[:, 0:128], rhs=dummy_sb[:],
                   start=True, stop=True)

            # --- startup DMAs: first wqk(0) quarter, then all of x(0)
            # (every unit streams all 8 x ct-chunks, so x gates the early
            # unit pipeline), then the remaining wqk(0) quarters.
            x0 = xpool.tile([128, 8 * M], BF, tag="x", name="x")
            x_sb[0] = x0
            for i in range(4):
                nc.sync.dma_start(
                    out=x0[:, i * 1024:(i + 1) * 1024],
                    in_=xiT[0:128, i * 1024:(i + 1) * 1024])
                nc.sync.dma_start(
                    out=wqk_sb[0][:, i * 1024:(i + 1) * 1024],
                    in_=wqk[0:128, i * 1024:(i + 1) * 1024])
            nc.sync.dma_start(out=bqk_sb[:], in_=bqk[:, :])
            nc.sync.dma_start(out=bout_sb[:], in_=bout[:, :])
            emit_w_qk_chunk(1)
            emit_w_qk_chunk(2)
            emit_w_qk_chunk(3)
            # wv lands after the qk chunks: QKV(0) consumes c4=2 at ~27us
            # but the v units not until ~40us -- emitting wv earlier
            # starved the qk stream mid-phase (HAM K=4 dip at 20-24us)
            emit_w_v()
            emit_x(1)
            emit_w_out()
            for u in range(20):
                emit_qkv_unit(0, u)
            # warm pair 0 of seg0: scores spread into the QKV tail
            emit_qkv_unit(0, 20)
            emit_qkv_unit(0, 21)
            warm0_front = emit_scores_front(0, 0)
            emit_qkv_unit(0, 22)
            warm0_pts = emit_scores_back(0, 0, warm0_front)
            emit_qkv_unit(0, 23)

            # Dense filler, 2 units per attention iteration, uniformly over
            # BOTH segments' attention phases so the PE span per pair-iter
            # (~5us) always exceeds the worst non-PE engine (~4.3us) and the
            # HAM never demotes.  Deadlines: seg1 v units before attn(1)'s
            # first PV; seg1 qk pair p before scores_pair(1, p) at iter p-1;
            # proj0 only after attn(0) is fully done.
            #   attn(0) iters 0-3: v units (16..23); iters 4-7: qk pairs 0-3
            #   attn(1) iters 0-3: qk pairs 4-7;     iters 4-7: proj0 tiles
            fill0_units = [16, 17, 18, 19, 20, 21, 22, 23,
                           0, 1, 2, 3, 4, 5, 6, 7]
            fill1_units = [8, 9, 10, 11, 12, 13, 14, 15]
            fill0 = iter(fill0_units)
            fill1 = iter(fill1_units)
            proj0 = iter(range(8))

            def fill_a0():
                for _ in range(2):
                    u = next(fill0, None)
                    if u is not None:
                        emit_qkv_unit(1, u)

            def fill_a1():
                for _ in range(2):
                    u = next(fill1, None)
                    if u is not None:
                        emit_qkv_unit(1, u)
                        continue
                    ot = next(proj0, None)
                    if ot is not None:
                        emit_proj_tile(0, ot)

            last0 = emit_attn(0, fill_a0, warm=warm0_pts)
            # warm-start seg1's pipeline the same way: its first pair's
            # score front runs before seg0's final PV pair, the back after
            warm1_front = emit_scores_front(1, 0)
            emit_pv_pair(0, last0[0], last0[1])
            warm1_pts = emit_scores_back(1, 0, warm1_front)
            last1 = emit_attn(1, fill_a1, warm=warm1_pts)
            emit_pv_pair(1, last1[0], last1[1])
            for ot in range(8):
                emit_proj_tile(1, ot)

    nc.finalize()
    return nc


def _prep_inputs(x, w_in, b_in, w_out, b_out):
    x = np.asarray(x, dtype=np.float32)
    w_in = np.asarray(w_in, dtype=np.float32)
    b_in = np.asarray(b_in, dtype=np.float32)
    w_out = np.asarray(w_out, dtype=np.float32)
    b_out = np.asarray(b_out, dtype=np.float32)

    # fold 1/sqrt(dh) into the q rows of w_in / b_in
    w_in_s = w_in.copy()
    b_in_s = b_in.copy()
    w_in_s[:C] *= DH ** -0.5
    b_in_s[:C] *= DH ** -0.5

    w_inT0 = np.ascontiguousarray(w_in_s.T).astype(BF16)
    # permute qk columns into [q_p | k_p] pairs matching the consume order
    w_inT = w_inT0.copy()
    for p in range(8):
        w_inT[:, p * 256:p * 256 + 128] = w_inT0[:, p * 128:(p + 1) * 128]
        w_inT[:, p * 256 + 128:(p + 1) * 256] = \
            w_inT0[:, C + p * 128:C + (p + 1) * 128]
    # repack into ct-major chunk layouts (one DMA per chunk on device)
    wp = w_inT.reshape(8, 128, 3 * C)
    wqk = np.ascontiguousarray(np.concatenate(
        [wp[:, :, c4 * 512:(c4 + 1) * 512].transpose(1, 0, 2).reshape(128, 8 * 512)
         for c4 in range(4)], axis=0))                       # (512, 4096)
    wv = np.ascontiguousarray(
        wp[:, :, 2 * C:].transpose(1, 0, 2).reshape(128, 8 * C))  # (128, 8192)
    w_outT = np.ascontiguousarray(w_out.T).astype(BF16)
    wout = np.ascontiguousarray(
        w_outT.reshape(8, 128, C).transpose(1, 0, 2).reshape(128, 8 * C))
    bqk = np.ascontiguousarray(b_in_s[:2 * C].reshape(16, 128).T, dtype=np.float32)
    # v bias folds exactly into an effective output bias:
    #   (p @ (v + 1 b_v^T)) / denom = (p @ v)/denom + b_v
    b_out_eff = b_out + w_out @ b_in[2 * C:]
    bout = np.ascontiguousarray(b_out_eff.reshape(8, 128).T, dtype=np.float32)

    # dilated gather + transpose + ct-major pack: per-core (2*128, 8*M)
    xi = x.reshape(B, S, W_SEG, C)[:, :, ::RATE, :]        # (B, S, M, C)
    xiT = np.ascontiguousarray(xi.transpose(0, 1, 3, 2)).astype(BF16)  # (B,S,C,M)
    xiT = xiT.reshape(16, 8, 128, M).transpose(0, 2, 1, 3)  # (16,128,8,M)
    xiT = np.ascontiguousarray(xiT).reshape(N_CORES, SEG_PER_CORE * 128, 8 * M)


    in_maps = []
    for c in range(N_CORES):
        in_maps.append({
            "xiT": np.ascontiguousarray(xiT[c]),
            "wqk": wqk,
            "wv": wv,
            "wout": wout,
            "bqk": bqk,
            "bout": bout,
        })
    return in_maps


def kernel(x, w_in, b_in, w_out, b_out, _trace=False):
    if "nc" not in _CACHE:
        _CACHE["nc"] = _build()
    nc = _CACHE["nc"]

    in_maps = _prep_inputs(x, w_in, b_in, w_out, b_out)
    res = run_bass_kernel_spmd(
        nc, in_maps, core_ids=list(range(N_CORES)), trace=_trace)
    _CACHE["last_result"] = res

    out = np.zeros((B, N, C), dtype=np.float32)
    ov = out.reshape(B, S, W_SEG, C)
    for c in range(N_CORES):
        yTc = np.asarray(res.results[c]["yT"], dtype=np.float32)
        a = yTc.reshape(SEG_PER_CORE, 4, 128, 2, M)
        for seg in range(SEG_PER_CORE):
            gseg = c * SEG_PER_CORE + seg
            b, s = divmod(gseg, S)
            yseg = a[seg].transpose(0, 2, 1, 3).reshape(C, M)
            ov[b, s, ::RATE, :] = yseg.T
    return out
